# revision 31
# baseline (speedup 1.0000x reference)
"""Trainium2 Bass kernel for nn_MHSA_40346922778634.

Math (per batch b, head h; the reference computes-then-drops the register
group, so reg_qk/reg_v are dead inputs):
  X = x[b] as [C=512, N=1024]
  Q = Wq X + bq ; K = Wk X + bk ; V = Wv X + bv       (per head: [64, N])
  P_h = (rel_h + rel_w) reshaped [head, 64, N]
  E[i,j] = Q_h[:,i].K_h[:,j] + P_h[:,i].Q_h[:,j]      ([N, N])
  attn = softmax(E, axis=-1)
  Out_h = V_h @ attn^T ; out[b, h*64:(h+1)*64] = Out_h + X[h*64:(h+1)*64]

Kernel strategy (8 cores, data-parallel over batch, 2 batches/core):
  - fp16 operands for projection + energy matmuls; bf16 for exp output
    (range) and AV matmuls.
  - Per head: z_h = [K_h; Q_h] produced DIRECTLY by the projection with a
    head-stacked weight layout (no partition-shuffle copies); u_h = [Q_h
    (DMA partition shift from z_h); P_h (DRAM)].
  - E^T = z^T u in one K=128 matmul per chunk; exp on ACT (the pacing
    engine at ~9.2us/head); AV = V_aug^T tt with a ones-column in vpt for
    the denominator row.
  - Softmax normalize: AV PSUM evacuated at once to SBUF (frees the PSUM
    slot fast -> avp bufs=1); denominator row packed [1,1024] -> [128,8]
    via DMA, reciprocal on all 128 DVE lanes (0.2us vs 6.5us), broadcast
    via DRAM staging + stride-0-partition DMA read (GpSimd
    partition_broadcast would swap Q7 libraries, ~7us/call), DVE multiply,
    GpSimd residual add (fp16 x).
  - Per-slot emission interleaves the lag-2 AV chunks + projections into
    the ACT-paced energy phase so the PE stream stays dense and warm;
    constants ride the scalar/gpsimd DMA queues so the sync queue and ACT
    queue are never head-of-line blocked.
"""

import sys

import numpy as np

try:
    import concourse.bass as bass  # noqa: F401
except Exception:  # pragma: no cover
    sys.path.insert(0, "/opt/trn_rl_repo")

import concourse.bass as bass  # noqa: F401
import concourse.tile as tile
from concourse import bacc, mybir
from concourse.bass_utils import run_bass_kernel_spmd

F32 = mybir.dt.float32
F16 = mybir.dt.float16
BF16 = mybir.dt.bfloat16
EXP = mybir.ActivationFunctionType.Exp

N_CORES = 8
B, C, WD, HD = 16, 512, 32, 32
HEAD, D, N = 8, 64, 1024
BPC = B // N_CORES  # batches per core
GH = BPC * HEAD  # global head slots per core


def build_bass():
    nc = bacc.Bacc("TRN2")

    xh_d = nc.dram_tensor("xh", [BPC, C, N], F16, kind="ExternalInput")
    wzt_d = nc.dram_tensor("wzt", [4, 128, 1024], F16, kind="ExternalInput")
    bz_d = nc.dram_tensor("bz", [HEAD, 128, 1], F32, kind="ExternalInput")
    wvpt_d = nc.dram_tensor("wvpt", [4, 128, 520], F16, kind="ExternalInput")
    bvp_d = nc.dram_tensor("bvp", [1, 520], F16, kind="ExternalInput")
    bvs_d = nc.dram_tensor("bvs", [16, 1], F32, kind="ExternalInput")
    wvs_d = nc.dram_tensor("wvs", [4, 128, 16], F16, kind="ExternalInput")
    pos_d = nc.dram_tensor("pos", [HEAD, D, N], F16, kind="ExternalInput")
    out_d = nc.dram_tensor("out", [BPC, C, N], F32, kind="ExternalOutput")
    # per-slot staging row for the reciprocal broadcast (DMA replicate)
    r_d = nc.dram_tensor("r_stage", [GH, N], BF16, kind="Internal")

    with tile.TileContext(nc) as tc:
        with (
            tc.tile_pool(name="consts", bufs=1) as cpool,
            tc.tile_pool(name="work", bufs=2) as wpool,
            tc.tile_pool(name="psum", bufs=2, space="PSUM") as ppool,
        ):
            # ---- constants (scalar HWDGE queue; x goes on sync queue) ----
            wzt_sb = cpool.tile([128, 4, 1024], F16, name="wzt_sb")
            for kc in range(4):
                nc.scalar.dma_start(wzt_sb[:, kc, 0:256], wzt_d[kc, :, 0:256])
            for kc in range(4):
                nc.scalar.dma_start(wzt_sb[:, kc, 256:1024], wzt_d[kc, :, 256:1024])
            # bz/wvpt/bvt ride the (startup-idle) GpSimd SWDGE queue so they
            # don't head-of-line block the ACT queue in front of the first
            # exps (only sync/scalar/gpsimd can issue DMAs).
            bz_sb = cpool.tile([128, HEAD], F32, name="bz_sb")
            for h in range(HEAD):
                nc.gpsimd.dma_start(bz_sb[:, h:h + 1], bz_d[h])
            wvpt_sb = cpool.tile([128, 4, 520], F16, name="wvpt_sb")
            for kc in range(4):
                nc.gpsimd.dma_start(wvpt_sb[:, kc, :], wvpt_d[kc])
            # V bias broadcast to all partitions once; added on DVE during the
            # PSUM->SBUF copy instead of via ones-row matmuls.
            bvt_sb = cpool.tile([128, 520], F16, name="bvt_sb")
            nc.gpsimd.dma_start(bvt_sb[:], bvp_d[0:1, :].broadcast_to([128, 520]))
            # strip bias as per-partition scalars (cols 512-519 + 8 pad
            # rows so the DMA transpose sees a 16-row source)
            bvs_sb = cpool.tile([16, 1], F32, name="bvs_sb")
            nc.gpsimd.dma_start(bvs_sb[:], bvs_d[:])
            wvs_sb = cpool.tile([128, 4, 16], F16, name="wvs_sb")
            for kc in range(4):
                nc.gpsimd.dma_start(wvs_sb[:, kc, :], wvs_d[kc])
            # warm the ACT exp table during the initial DMAs
            warm_in = cpool.tile([1, 8], F32, name="warm_in")
            nc.vector.memset(warm_in[:], 0.0)
            warm_out = cpool.tile([1, 8], F32, name="warm_out")
            nc.scalar.activation(warm_out[:], warm_in[:], EXP)
            # u_all: per-head [Q_h(b); P_h] — P halves (partitions 64-127)
            # are batch-invariant, loaded once here.
            u_all = cpool.tile([128, HEAD, N], F16, name="u_all")
            for h in range(2):
                nc.sync.dma_start(u_all[64:128, h, :], pos_d[h])
            for h in range(2, HEAD):
                nc.gpsimd.dma_start(u_all[64:128, h, :], pos_d[h])

            def prep_x(b, x_sb=None, halves=(0, 1)):
                # DMA x on the sync queue (scalar-queue DMAs steal ACT time).
                if x_sb is None:
                    x_sb = wpool.tile([128, 4, N], F16, name=f"x_{b}", tag="x")
                for nh in halves:
                    for kc in range(4):
                        nc.sync.dma_start(
                            x_sb[:, kc, nh * 512:(nh + 1) * 512],
                            xh_d[b, kc * 128:(kc + 1) * 128, nh * 512:(nh + 1) * 512],
                        )
                return x_sb

            def emit_zproj_half(x_sb, h, z, nh):
                ps = ppool.tile([128, 512], F32, name=f"ps_z{h}{nh}", tag="pse", bufs=3)
                for kc in range(4):
                    nc.tensor.matmul(
                        ps[:],
                        wzt_sb[:, kc, h * 128:(h + 1) * 128],
                        x_sb[:, kc, nh * 512:(nh + 1) * 512],
                        start=(kc == 0),
                        stop=(kc == 3),
                    )
                nc.vector.tensor_scalar_add(
                    z[:, nh * 512:(nh + 1) * 512], ps[:], bz_sb[:, h:h + 1]
                )

            def emit_zproj(x_sb, h):
                # z_h = [K_h; Q_h] stacked on partitions, bias added, f16.
                z = wpool.tile([128, N], F16, name=f"z_{h}", tag="z", bufs=4)
                for nh in range(2):
                    emit_zproj_half(x_sb, h, z, nh)
                return z

            def emit_u(z, h):
                # u_h rows 0-63 = Q_h (partition-shift copy from z rows
                # 64-127); rows 64-127 (P_h) were loaded once at startup.
                nc.sync.dma_start(u_all[0:64, h, :], z[64:128, :])

            def emit_vproj(x_sb, vpt, c0, c1):
                # V^T projection cols 0-511 (bf16); bias added on DVE during
                # PSUM evacuation.
                for c8 in range(c0, c1):
                    ps = ppool.tile([128, 512], F32, name=f"ps_v{c8}", tag="pse", bufs=3)
                    for kc in range(4):
                        nc.tensor.matmul(
                            ps[:],
                            x_sb[:, kc, c8 * 128:(c8 + 1) * 128],
                            wvpt_sb[:, kc, 0:512],
                            start=(kc == 0),
                            stop=(kc == 3),
                        )
                    nc.vector.tensor_add(vpt[:, c8, 0:512], ps[:], bvt_sb[:, 0:512])

            def emit_vstrip(x_sb, vpt, b):
                # cols 512-519 computed transposed (8 rows x 1024 positions:
                # two N=512 matmuls x 4 kc instead of 32 LDW-bound N=8
                # matmuls), bias per-partition, then DMA-transposed into vpt.
                ps = ppool.tile([16, N], F32, name=f"ps_vs{b}", tag="pse", bufs=3)
                for ih in range(2):
                    for kc in range(4):
                        nc.tensor.matmul(
                            ps[:, ih * 512:(ih + 1) * 512],
                            wvs_sb[:, kc, :],
                            x_sb[:, kc, ih * 512:(ih + 1) * 512],
                            start=(kc == 0),
                            stop=(kc == 3),
                        )
                sb16 = wpool.tile([16, N], BF16, name=f"vs_{b}", tag="vs")
                nc.vector.tensor_scalar_add(sb16[:], ps[:], bvs_sb[:])
                # one consecutive block of transpose DMAs (xbar-mode switch
                # serializes the queue per transition, so batch them)
                for c8 in range(8):
                    nc.sync.dma_start(
                        vpt[:, c8, 512:528],
                        sb16[:, c8 * 128:(c8 + 1) * 128],
                        transpose=True,
                    )

            def emit_E_chunks(z, h, gh):
                # E^T chunks + exp; ACT paces this phase.
                tts = []
                for j in range(8):
                    eps = ppool.tile([128, N], F32, name=f"ps_e{gh}{j}", tag="pse", bufs=3)
                    for ih in range(2):
                        nc.tensor.matmul(
                            eps[:, ih * 512:(ih + 1) * 512],
                            z[:, j * 128:(j + 1) * 128],
                            u_all[:, h, ih * 512:(ih + 1) * 512],
                            start=True,
                            stop=True,
                        )
                    tt = wpool.tile([128, N], BF16, name=f"tt_{gh}_{j}", tag="tt", bufs=22)
                    nc.scalar.activation(tt[:], eps[:], EXP)
                    tts.append(tt)
                return tts

            def emit_AV(vpt, h, tts, gh):
                ops = ppool.tile([65, N], F32, name=f"ps_o{gh}", tag="avp", bufs=1)
                for j in range(8):
                    for mh in range(2):
                        nc.tensor.matmul(
                            ops[:, mh * 512:(mh + 1) * 512],
                            vpt[:, j, h * 65:h * 65 + 65],
                            tts[j][:, mh * 512:(mh + 1) * 512],
                            start=(j == 0),
                            stop=(j == 7),
                        )
                # evacuate PSUM immediately: frees the avp slot ~1.5us after
                # the burst so the next slot's AV never waits on the (long)
                # normalize chain; row 64 doubles as the denominator row.
                oc = wpool.tile([65, N], F32, name=f"oc_{gh}", tag="oc", bufs=4)
                nc.vector.tensor_copy(oc[:], ops[:])
                return oc

            def emit_norm_head(b, h, oc, gh):
                # DMA-only part of the normalize, emitted right after the AV
                # PSUM evacuation: residual prefetch + denominator pack.
                xres = wpool.tile([64, N], F16, name=f"xres_{gh}", tag="xres", bufs=4)
                nc.sync.dma_start(xres[:], xh_d[b, h * 64:(h + 1) * 64, :])
                dp = wpool.tile([128, 8], F32, name=f"dp_{gh}", tag="dp", bufs=3)
                nc.sync.dma_start(dp[:], oc[64:65, :])
                return {"b": b, "h": h, "oc": oc, "xres": xres, "dp": dp, "gh": gh}

            def emit_norm_tail(st):
                # compute part, emitted a slot later so the DVE queue head
                # never waits on the pack DMA (which blocked the next slot's
                # bias-adds and stalled projection PSUM recycling).
                gh, b, h = st["gh"], st["b"], st["h"]
                rp = wpool.tile([128, 8], F32, name=f"rp_{gh}", tag="rp", bufs=3)
                nc.vector.reciprocal(rp[:], st["dp"][:])
                rpb = wpool.tile([128, 8], BF16, name=f"rpb_{gh}", tag="rpb", bufs=3)
                nc.vector.tensor_copy(rpb[:], rp[:])
                # broadcast 1/denom to 64 partitions via DRAM staging + a
                # stride-0-partition DMA read (GpSimd partition_broadcast
                # forces a Q7 library swap per call — ~7us stall)
                nc.sync.dma_start(r_d[gh:gh + 1, :], rpb[:])
                rps = wpool.tile([64, N], BF16, name=f"rps_{gh}", tag="rps", bufs=3)
                nc.sync.dma_start(rps[:], r_d[gh:gh + 1, :].broadcast_to([64, N]))
                osb = wpool.tile([64, N], F32, name=f"osb_{gh}", tag="osb", bufs=4)
                nc.vector.tensor_mul(osb[:], st["oc"][0:64, :], rps[:])
                fin = wpool.tile([64, N], F32, name=f"fin_{gh}", tag="fin", bufs=4)
                nc.vector.tensor_add(fin[:], osb[:], st["xres"][:])
                nc.sync.dma_start(out_d[b, h * 64:(h + 1) * 64, :], fin[:])

            # ---- software pipeline over GH=16 global head slots ----
            xs = {0: prep_x(0)}
            vpts = {0: wpool.tile([128, 8, 528], BF16, name="vpt_0", tag="vpt")}
            Z, TT, OPS = {}, {}, {}
            Z[0] = emit_zproj(xs[0], 0)
            emit_u(Z[0], 0)
            Z[1] = emit_zproj(xs[0], 1)
            emit_u(Z[1], 1)

            NORM = {}
            for gh in range(GH):
                b, h = divmod(gh, HEAD)
                z = Z.pop(gh)
                # normalize compute for slot gh-3 first: its DMAs landed a
                # slot ago, so the DVE queue head never blocks on them.
                if gh - 3 in NORM:
                    emit_norm_tail(NORM.pop(gh - 3))
                # Build PE "filler" units to interleave into the ACT-paced
                # energy phase so the PE never idles waiting on exp.
                fillers = []
                av_state = {}
                if gh >= 2:
                    b2, h2 = divmod(gh - 2, HEAD)
                    tts2 = TT.pop(gh - 2)
                    ops2 = ppool.tile(
                        [65, N], F32, name=f"ps_o{gh - 2}", tag="avp", bufs=1
                    )
                    av_state = {"ops": ops2, "b2": b2, "h2": h2}

                    def av_unit(j, ops2=ops2, vpt=vpts[b2], h2=h2, tts2=tts2):
                        for mh in range(2):
                            nc.tensor.matmul(
                                ops2[:, mh * 512:(mh + 1) * 512],
                                vpt[:, j, h2 * 65:h2 * 65 + 65],
                                tts2[j][:, mh * 512:(mh + 1) * 512],
                                start=(j == 0),
                                stop=(j == 7),
                            )

                    fillers += [lambda j=j: av_unit(j) for j in range(8)]
                if gh + 2 < GH:
                    b3, h3 = divmod(gh + 2, HEAD)
                    z3 = wpool.tile([128, N], F16, name=f"z_{gh + 2}", tag="z", bufs=4)
                    Z[gh + 2] = z3
                    fillers += [
                        lambda nh=nh, z3=z3, b3=b3, h3=h3: emit_zproj_half(
                            xs[b3], h3, z3, nh
                        )
                        for nh in range(2)
                    ]
                if gh < 2:
                    fillers += [
                        lambda c=c: emit_vproj(xs[0], vpts[0], c, c + 1)
                        for c in range(4 * gh, 4 * gh + 4)
                    ]
                elif 6 <= gh <= 9:
                    fillers += [
                        lambda c=c: emit_vproj(xs[1], vpts[1], c, c + 1)
                        for c in range(2 * (gh - 6), 2 * (gh - 6) + 2)
                    ]
                if gh == 1:
                    fillers.append(lambda: emit_vstrip(xs[0], vpts[0], 0))
                elif gh == 9:
                    fillers.append(lambda: emit_vstrip(xs[1], vpts[1], 1))
                if gh == 4:
                    xs[1] = prep_x(1, halves=(0,))
                elif gh == 5:
                    prep_x(1, x_sb=xs[1], halves=(1,))
                    vpts[1] = wpool.tile([128, 8, 528], BF16, name="vpt_1", tag="vpt")

                # energy + exp for slot gh, fillers interleaved
                tts, fi = [], 0
                for j in range(8):
                    eps = ppool.tile([128, N], F32, name=f"ps_e{gh}{j}", tag="pse", bufs=3)
                    for ih in range(2):
                        nc.tensor.matmul(
                            eps[:, ih * 512:(ih + 1) * 512],
                            z[:, j * 128:(j + 1) * 128],
                            u_all[:, h, ih * 512:(ih + 1) * 512],
                            start=True,
                            stop=True,
                        )
                    tt = wpool.tile([128, N], BF16, name=f"tt_{gh}_{j}", tag="tt", bufs=22)
                    nc.scalar.activation(tt[:], eps[:], EXP)
                    tts.append(tt)
                    if j >= 1:
                        for _ in range(2):
                            if fi < len(fillers):
                                fillers[fi]()
                                fi += 1
                while fi < len(fillers):
                    fillers[fi]()
                    fi += 1
                TT[gh] = tts
                if gh + 2 < GH:
                    emit_u(Z[gh + 2], (gh + 2) % HEAD)
                if av_state:
                    ops2 = av_state["ops"]
                    oc = wpool.tile(
                        [65, N], F32, name=f"oc_{gh - 2}", tag="oc", bufs=4
                    )
                    nc.vector.tensor_copy(oc[:], ops2[:])
                    NORM[gh - 2] = emit_norm_head(
                        av_state["b2"], av_state["h2"], oc, gh - 2
                    )

            # epilogue: AV + norm for the last two slots; the final AV uses
            # a pse-tag PSUM tile (energy psum is drained by then) so it does
            # not serialize on the avp slot release.
            for gh, tag in ((GH - 2, "avp"), (GH - 1, "pse")):
                b2, h2 = divmod(gh, HEAD)
                tts2 = TT.pop(gh)
                ops2 = ppool.tile([65, N], F32, name=f"ps_o{gh}", tag=tag,
                                  bufs=1 if tag == "avp" else 3)
                for j in range(8):
                    for mh in range(2):
                        nc.tensor.matmul(
                            ops2[:, mh * 512:(mh + 1) * 512],
                            vpts[b2][:, j, h2 * 65:h2 * 65 + 65],
                            tts2[j][:, mh * 512:(mh + 1) * 512],
                            start=(j == 0),
                            stop=(j == 7),
                        )
                oc = wpool.tile([65, N], F32, name=f"oc_{gh}", tag="oc", bufs=4)
                nc.vector.tensor_copy(oc[:], ops2[:])
                NORM[gh] = emit_norm_head(b2, h2, oc, gh)
                if gh - 1 in NORM:
                    emit_norm_tail(NORM.pop(gh - 1))
            for k in sorted(NORM):
                emit_norm_tail(NORM.pop(k))

    nc.compile()
    return nc


def _prep_consts(Wq, bq, Wk, bk, Wv, bv, rel_h, rel_w):
    WkT = np.ascontiguousarray(Wk.T).reshape(4, 128, 512)
    WqT = np.ascontiguousarray(Wq.T).reshape(4, 128, 512)
    wzt = np.empty((4, 128, 1024), np.float32)
    bz = np.empty((HEAD, 128, 1), np.float32)
    for h in range(HEAD):
        wzt[:, :, h * 128:h * 128 + 64] = WkT[:, :, h * 64:(h + 1) * 64]
        wzt[:, :, h * 128 + 64:h * 128 + 128] = WqT[:, :, h * 64:(h + 1) * 64]
        bz[h, 0:64, 0] = bk[h * 64:(h + 1) * 64]
        bz[h, 64:128, 0] = bq[h * 64:(h + 1) * 64]
    wvpt = np.zeros((512, 520), np.float32)
    bvp = np.zeros((1, 520), np.float32)
    for h in range(HEAD):
        wvpt[:, h * 65:h * 65 + 64] = Wv[h * 64:(h + 1) * 64, :].T
        bvp[0, h * 65:h * 65 + 64] = bv[h * 64:(h + 1) * 64]
        bvp[0, h * 65 + 64] = 1.0
    pos = (rel_h + rel_w).reshape(HEAD, D, N).astype(np.float16)
    return {
        "wzt": wzt.astype(np.float16),
        "bz": bz,
        "wvpt": wvpt.reshape(4, 128, 520).astype(np.float16),
        "bvp": bvp.astype(np.float16),
        "bvs": np.concatenate([bvp[0, 512:520], np.zeros(8, np.float32)]).reshape(16, 1).astype(np.float32),
        "wvs": np.concatenate(
            [wvpt.reshape(4, 128, 520)[:, :, 512:520],
             np.zeros((4, 128, 8), np.float32)], axis=2).astype(np.float16),
        "pos": pos,
    }


_CACHE = {}


def build_in_maps(x, Wq, bq, Wk, bk, Wv, bv, rel_h, rel_w):
    x = np.asarray(x, np.float32)
    consts = _prep_consts(
        *[np.asarray(a, np.float32) for a in (Wq, bq, Wk, bk, Wv, bv, rel_h, rel_w)]
    )
    xh = x.reshape(B, C, N).astype(np.float16)
    in_maps = []
    for c in range(N_CORES):
        m = dict(consts)
        m["xh"] = np.ascontiguousarray(xh[c * BPC:(c + 1) * BPC])
        in_maps.append(m)
    return in_maps


def kernel(x, Wq, bq, Wk, bk, Wv, bv, rel_h, rel_w, reg_qk, reg_v):
    # reg_qk / reg_v are computed-then-dropped by the reference -> unused.
    in_maps = build_in_maps(x, Wq, bq, Wk, bk, Wv, bv, rel_h, rel_w)

    if "nc" not in _CACHE:
        _CACHE["nc"] = build_bass()
    res = run_bass_kernel_spmd(_CACHE["nc"], in_maps, list(range(N_CORES)))
    outs = [np.asarray(r["out"]) for r in res.results]
    return np.concatenate(outs, axis=0).reshape(B, C, WD, HD)


if __name__ == "__main__":
    nc = build_bass()
    print("built ok")


# revision 32
# speedup vs baseline: 1.0818x; 1.0818x over previous
"""Trainium2 Bass kernel for nn_MHSA_40346922778634.

Math (per batch b, head h; the reference computes-then-drops the register
group, so reg_qk/reg_v are dead inputs):
  X = x[b] as [C=512, N=1024]
  Q = Wq X + bq ; K = Wk X + bk ; V = Wv X + bv       (per head: [64, N])
  P_h = (rel_h + rel_w) reshaped [head, 64, N]
  E[i,j] = Q_h[:,i].K_h[:,j] + P_h[:,i].Q_h[:,j]      ([N, N])
  attn = softmax(E, axis=-1)
  Out_h = V_h @ attn^T ; out[b, h*64:(h+1)*64] = Out_h + X[h*64:(h+1)*64]

Kernel strategy (8 cores, data-parallel over batch, 2 batches/core):
  - fp16 operands for projection + energy matmuls; bf16 for exp output
    (range) and AV matmuls.
  - Per head: z_h = [K_h; Q_h] produced DIRECTLY by the projection with a
    head-stacked weight layout (no partition-shuffle copies); u_h = [Q_h
    (DMA partition shift from z_h); P_h (DRAM)].
  - E^T = z^T u in one K=128 matmul per chunk; exp on ACT (the pacing
    engine at ~9.2us/head); AV = V_aug^T tt with a ones-column in vpt for
    the denominator row.
  - Softmax normalize: AV PSUM evacuated at once to SBUF (frees the PSUM
    slot fast -> avp bufs=1); denominator row packed [1,1024] -> [128,8]
    via DMA, reciprocal on all 128 DVE lanes (0.2us vs 6.5us), broadcast
    via DRAM staging + stride-0-partition DMA read (GpSimd
    partition_broadcast would swap Q7 libraries, ~7us/call), DVE multiply,
    GpSimd residual add (fp16 x).
  - Per-slot emission interleaves the lag-2 AV chunks + projections into
    the ACT-paced energy phase so the PE stream stays dense and warm;
    constants ride the scalar/gpsimd DMA queues so the sync queue and ACT
    queue are never head-of-line blocked.
"""

import sys

import numpy as np

try:
    import concourse.bass as bass  # noqa: F401
except Exception:  # pragma: no cover
    sys.path.insert(0, "/opt/trn_rl_repo")

import concourse.bass as bass  # noqa: F401
import concourse.tile as tile
from concourse import bacc, mybir
from concourse.bass_utils import run_bass_kernel_spmd

F32 = mybir.dt.float32
F16 = mybir.dt.float16
BF16 = mybir.dt.bfloat16
EXP = mybir.ActivationFunctionType.Exp

N_CORES = 8
B, C, WD, HD = 16, 512, 32, 32
HEAD, D, N = 8, 64, 1024
BPC = B // N_CORES  # batches per core
GH = BPC * HEAD  # global head slots per core


def build_bass():
    nc = bacc.Bacc("TRN2")

    xh_d = nc.dram_tensor("xh", [BPC, C, N], F16, kind="ExternalInput")
    wzt_d = nc.dram_tensor("wzt", [4, 128, 1024], F16, kind="ExternalInput")
    bz_d = nc.dram_tensor("bz", [HEAD, 128, 1], F32, kind="ExternalInput")
    wvpt_d = nc.dram_tensor("wvpt", [4, 128, 520], F16, kind="ExternalInput")
    bvp_d = nc.dram_tensor("bvp", [1, 520], F16, kind="ExternalInput")
    pos_d = nc.dram_tensor("pos", [HEAD, D, N], F16, kind="ExternalInput")
    out_d = nc.dram_tensor("out", [BPC, C, N], F32, kind="ExternalOutput")
    # per-slot staging row for the reciprocal broadcast (DMA replicate)
    r_d = nc.dram_tensor("r_stage", [GH, N], BF16, kind="Internal")

    with tile.TileContext(nc) as tc:
        with (
            tc.tile_pool(name="consts", bufs=1) as cpool,
            tc.tile_pool(name="work", bufs=2) as wpool,
            tc.tile_pool(name="psum", bufs=2, space="PSUM") as ppool,
        ):
            # ---- constants (scalar HWDGE queue; x goes on sync queue) ----
            wzt_sb = cpool.tile([128, 4, 1024], F16, name="wzt_sb")
            for kc in range(4):
                nc.scalar.dma_start(wzt_sb[:, kc, 0:256], wzt_d[kc, :, 0:256])
            for kc in range(4):
                nc.scalar.dma_start(wzt_sb[:, kc, 256:1024], wzt_d[kc, :, 256:1024])
            # bz/wvpt/bvt ride the (startup-idle) GpSimd SWDGE queue so they
            # don't head-of-line block the ACT queue in front of the first
            # exps (only sync/scalar/gpsimd can issue DMAs).
            bz_sb = cpool.tile([128, HEAD], F32, name="bz_sb")
            for h in range(HEAD):
                nc.gpsimd.dma_start(bz_sb[:, h:h + 1], bz_d[h])
            wvpt_sb = cpool.tile([128, 4, 520], F16, name="wvpt_sb")
            for kc in range(4):
                nc.gpsimd.dma_start(wvpt_sb[:, kc, :], wvpt_d[kc])
            # V bias broadcast to all partitions once; added on DVE during the
            # PSUM->SBUF copy instead of via ones-row matmuls.
            bvt_sb = cpool.tile([128, 520], F16, name="bvt_sb")
            nc.gpsimd.dma_start(bvt_sb[:], bvp_d[0:1, :].broadcast_to([128, 520]))
            # warm the ACT exp table during the initial DMAs
            warm_in = cpool.tile([1, 8], F32, name="warm_in")
            nc.vector.memset(warm_in[:], 0.0)
            warm_out = cpool.tile([1, 8], F32, name="warm_out")
            nc.scalar.activation(warm_out[:], warm_in[:], EXP)
            # u_all: per-head [Q_h(b); P_h] — P halves (partitions 64-127)
            # are batch-invariant, loaded once here.
            u_all = cpool.tile([128, HEAD, N], F16, name="u_all")
            for h in range(2):
                nc.sync.dma_start(u_all[64:128, h, :], pos_d[h])
            for h in range(2, HEAD):
                nc.gpsimd.dma_start(u_all[64:128, h, :], pos_d[h])

            def prep_x(b, x_sb=None, halves=(0, 1)):
                # DMA x on the sync queue (scalar-queue DMAs steal ACT time).
                if x_sb is None:
                    x_sb = wpool.tile([128, 4, N], F16, name=f"x_{b}", tag="x")
                for nh in halves:
                    for kc in range(4):
                        nc.sync.dma_start(
                            x_sb[:, kc, nh * 512:(nh + 1) * 512],
                            xh_d[b, kc * 128:(kc + 1) * 128, nh * 512:(nh + 1) * 512],
                        )
                return x_sb

            def emit_zproj_half(x_sb, h, z, nh):
                ps = ppool.tile([128, 512], F32, name=f"ps_z{h}{nh}", tag="pse", bufs=3)
                for kc in range(4):
                    nc.tensor.matmul(
                        ps[:],
                        wzt_sb[:, kc, h * 128:(h + 1) * 128],
                        x_sb[:, kc, nh * 512:(nh + 1) * 512],
                        start=(kc == 0),
                        stop=(kc == 3),
                    )
                nc.vector.tensor_scalar_add(
                    z[:, nh * 512:(nh + 1) * 512], ps[:], bz_sb[:, h:h + 1]
                )

            def emit_zproj(x_sb, h):
                # z_h = [K_h; Q_h] stacked on partitions, bias added, f16.
                z = wpool.tile([128, N], F16, name=f"z_{h}", tag="z", bufs=4)
                for nh in range(2):
                    emit_zproj_half(x_sb, h, z, nh)
                return z

            def emit_u(z, h):
                # u_h rows 0-63 = Q_h (partition-shift copy from z rows
                # 64-127); rows 64-127 (P_h) were loaded once at startup.
                nc.sync.dma_start(u_all[0:64, h, :], z[64:128, :])

            def emit_vproj(x_sb, vpt, c0, c1):
                # V^T padded projection (bf16); bias (incl. the ones column)
                # is added by the DVE during PSUM evacuation.
                for c8 in range(c0, c1):
                    ps = ppool.tile([128, 520], F32, name=f"ps_v{c8}", tag="pse", bufs=3)
                    for (lo, hi) in ((0, 512), (512, 520)):
                        for kc in range(4):
                            nc.tensor.matmul(
                                ps[:, lo:hi],
                                x_sb[:, kc, c8 * 128:(c8 + 1) * 128],
                                wvpt_sb[:, kc, lo:hi],
                                start=(kc == 0),
                                stop=(kc == 3),
                            )
                    nc.vector.tensor_add(vpt[:, c8, :], ps[:], bvt_sb[:])

            def emit_E_chunks(z, h, gh):
                # E^T chunks + exp; ACT paces this phase.
                tts = []
                for j in range(8):
                    eps = ppool.tile([128, N], F32, name=f"ps_e{gh}{j}", tag="pse", bufs=3)
                    for ih in range(2):
                        nc.tensor.matmul(
                            eps[:, ih * 512:(ih + 1) * 512],
                            z[:, j * 128:(j + 1) * 128],
                            u_all[:, h, ih * 512:(ih + 1) * 512],
                            start=True,
                            stop=True,
                        )
                    tt = wpool.tile([128, N], BF16, name=f"tt_{gh}_{j}", tag="tt", bufs=22)
                    nc.scalar.activation(tt[:], eps[:], EXP)
                    tts.append(tt)
                return tts

            def emit_AV(vpt, h, tts, gh):
                ops = ppool.tile([65, N], F32, name=f"ps_o{gh}", tag="avp", bufs=1)
                for j in range(8):
                    for mh in range(2):
                        nc.tensor.matmul(
                            ops[:, mh * 512:(mh + 1) * 512],
                            vpt[:, j, h * 65:h * 65 + 65],
                            tts[j][:, mh * 512:(mh + 1) * 512],
                            start=(j == 0),
                            stop=(j == 7),
                        )
                # evacuate PSUM immediately: frees the avp slot ~1.5us after
                # the burst so the next slot's AV never waits on the (long)
                # normalize chain; row 64 doubles as the denominator row.
                oc = wpool.tile([65, N], F32, name=f"oc_{gh}", tag="oc", bufs=4)
                nc.vector.tensor_copy(oc[:], ops[:])
                return oc

            def emit_norm_head(b, h, oc, gh):
                # DMA-only part of the normalize, emitted right after the AV
                # PSUM evacuation: residual prefetch + denominator pack.
                xres = wpool.tile([64, N], F16, name=f"xres_{gh}", tag="xres", bufs=4)
                nc.sync.dma_start(xres[:], xh_d[b, h * 64:(h + 1) * 64, :])
                dp = wpool.tile([128, 8], F32, name=f"dp_{gh}", tag="dp", bufs=3)
                nc.sync.dma_start(dp[:], oc[64:65, :])
                return {"b": b, "h": h, "oc": oc, "xres": xres, "dp": dp, "gh": gh}

            def emit_norm_tail(st):
                # compute part, emitted a slot later so the DVE queue head
                # never waits on the pack DMA (which blocked the next slot's
                # bias-adds and stalled projection PSUM recycling).
                gh, b, h = st["gh"], st["b"], st["h"]
                rp = wpool.tile([128, 8], F32, name=f"rp_{gh}", tag="rp", bufs=3)
                nc.vector.reciprocal(rp[:], st["dp"][:])
                rpb = wpool.tile([128, 8], BF16, name=f"rpb_{gh}", tag="rpb", bufs=3)
                nc.vector.tensor_copy(rpb[:], rp[:])
                # broadcast 1/denom to 64 partitions via DRAM staging + a
                # stride-0-partition DMA read (GpSimd partition_broadcast
                # forces a Q7 library swap per call — ~7us stall)
                nc.sync.dma_start(r_d[gh:gh + 1, :], rpb[:])
                rps = wpool.tile([64, N], BF16, name=f"rps_{gh}", tag="rps", bufs=3)
                nc.sync.dma_start(rps[:], r_d[gh:gh + 1, :].broadcast_to([64, N]))
                osb = wpool.tile([64, N], F32, name=f"osb_{gh}", tag="osb", bufs=4)
                nc.vector.tensor_mul(osb[:], st["oc"][0:64, :], rps[:])
                fin = wpool.tile([64, N], F32, name=f"fin_{gh}", tag="fin", bufs=4)
                nc.vector.tensor_add(fin[:], osb[:], st["xres"][:])
                nc.sync.dma_start(out_d[b, h * 64:(h + 1) * 64, :], fin[:])

            # ---- software pipeline over GH=16 global head slots ----
            xs = {0: prep_x(0)}
            vpts = {0: wpool.tile([128, 8, 520], BF16, name="vpt_0", tag="vpt")}
            Z, TT, OPS = {}, {}, {}
            Z[0] = emit_zproj(xs[0], 0)
            emit_u(Z[0], 0)
            Z[1] = emit_zproj(xs[0], 1)
            emit_u(Z[1], 1)

            NORM = {}
            for gh in range(GH):
                b, h = divmod(gh, HEAD)
                z = Z.pop(gh)
                # normalize compute for slot gh-3 first: its DMAs landed a
                # slot ago, so the DVE queue head never blocks on them.
                if gh - 3 in NORM:
                    emit_norm_tail(NORM.pop(gh - 3))
                # Build PE "filler" units to interleave into the ACT-paced
                # energy phase so the PE never idles waiting on exp.
                fillers = []
                av_state = {}
                if gh >= 2:
                    b2, h2 = divmod(gh - 2, HEAD)
                    tts2 = TT.pop(gh - 2)
                    ops2 = ppool.tile(
                        [65, N], F32, name=f"ps_o{gh - 2}", tag="avp", bufs=1
                    )
                    av_state = {"ops": ops2, "b2": b2, "h2": h2}

                    def av_unit(j, ops2=ops2, vpt=vpts[b2], h2=h2, tts2=tts2):
                        for mh in range(2):
                            nc.tensor.matmul(
                                ops2[:, mh * 512:(mh + 1) * 512],
                                vpt[:, j, h2 * 65:h2 * 65 + 65],
                                tts2[j][:, mh * 512:(mh + 1) * 512],
                                start=(j == 0),
                                stop=(j == 7),
                            )

                    fillers += [lambda j=j: av_unit(j) for j in range(8)]
                if gh + 2 < GH:
                    b3, h3 = divmod(gh + 2, HEAD)
                    z3 = wpool.tile([128, N], F16, name=f"z_{gh + 2}", tag="z", bufs=4)
                    Z[gh + 2] = z3
                    fillers += [
                        lambda nh=nh, z3=z3, b3=b3, h3=h3: emit_zproj_half(
                            xs[b3], h3, z3, nh
                        )
                        for nh in range(2)
                    ]
                if gh < 2:
                    fillers += [
                        lambda c=c: emit_vproj(xs[0], vpts[0], c, c + 1)
                        for c in range(4 * gh, 4 * gh + 4)
                    ]
                elif 6 <= gh <= 9:
                    fillers += [
                        lambda c=c: emit_vproj(xs[1], vpts[1], c, c + 1)
                        for c in range(2 * (gh - 6), 2 * (gh - 6) + 2)
                    ]
                if gh == 4:
                    xs[1] = prep_x(1, halves=(0,))
                elif gh == 5:
                    prep_x(1, x_sb=xs[1], halves=(1,))
                    vpts[1] = wpool.tile([128, 8, 520], BF16, name="vpt_1", tag="vpt")

                # energy + exp for slot gh, fillers interleaved
                tts, fi = [], 0
                for j in range(8):
                    eps = ppool.tile([128, N], F32, name=f"ps_e{gh}{j}", tag="pse", bufs=3)
                    for ih in range(2):
                        nc.tensor.matmul(
                            eps[:, ih * 512:(ih + 1) * 512],
                            z[:, j * 128:(j + 1) * 128],
                            u_all[:, h, ih * 512:(ih + 1) * 512],
                            start=True,
                            stop=True,
                        )
                    tt = wpool.tile([128, N], BF16, name=f"tt_{gh}_{j}", tag="tt", bufs=22)
                    nc.scalar.activation(tt[:], eps[:], EXP)
                    tts.append(tt)
                    if j >= 1:
                        for _ in range(2):
                            if fi < len(fillers):
                                fillers[fi]()
                                fi += 1
                while fi < len(fillers):
                    fillers[fi]()
                    fi += 1
                TT[gh] = tts
                if gh + 2 < GH:
                    emit_u(Z[gh + 2], (gh + 2) % HEAD)
                if av_state:
                    ops2 = av_state["ops"]
                    oc = wpool.tile(
                        [65, N], F32, name=f"oc_{gh - 2}", tag="oc", bufs=4
                    )
                    nc.vector.tensor_copy(oc[:], ops2[:])
                    NORM[gh - 2] = emit_norm_head(
                        av_state["b2"], av_state["h2"], oc, gh - 2
                    )

            # epilogue: AV + norm for the last two slots; the final AV uses
            # a pse-tag PSUM tile (energy psum is drained by then) so it does
            # not serialize on the avp slot release.
            for gh, tag in ((GH - 2, "avp"), (GH - 1, "pse")):
                b2, h2 = divmod(gh, HEAD)
                tts2 = TT.pop(gh)
                ops2 = ppool.tile([65, N], F32, name=f"ps_o{gh}", tag=tag,
                                  bufs=1 if tag == "avp" else 3)
                for j in range(8):
                    for mh in range(2):
                        nc.tensor.matmul(
                            ops2[:, mh * 512:(mh + 1) * 512],
                            vpts[b2][:, j, h2 * 65:h2 * 65 + 65],
                            tts2[j][:, mh * 512:(mh + 1) * 512],
                            start=(j == 0),
                            stop=(j == 7),
                        )
                oc = wpool.tile([65, N], F32, name=f"oc_{gh}", tag="oc", bufs=4)
                nc.vector.tensor_copy(oc[:], ops2[:])
                NORM[gh] = emit_norm_head(b2, h2, oc, gh)
                if gh - 1 in NORM:
                    emit_norm_tail(NORM.pop(gh - 1))
            for k in sorted(NORM):
                emit_norm_tail(NORM.pop(k))

    nc.compile()
    return nc


def _prep_consts(Wq, bq, Wk, bk, Wv, bv, rel_h, rel_w):
    WkT = np.ascontiguousarray(Wk.T).reshape(4, 128, 512)
    WqT = np.ascontiguousarray(Wq.T).reshape(4, 128, 512)
    wzt = np.empty((4, 128, 1024), np.float32)
    bz = np.empty((HEAD, 128, 1), np.float32)
    for h in range(HEAD):
        wzt[:, :, h * 128:h * 128 + 64] = WkT[:, :, h * 64:(h + 1) * 64]
        wzt[:, :, h * 128 + 64:h * 128 + 128] = WqT[:, :, h * 64:(h + 1) * 64]
        bz[h, 0:64, 0] = bk[h * 64:(h + 1) * 64]
        bz[h, 64:128, 0] = bq[h * 64:(h + 1) * 64]
    wvpt = np.zeros((512, 520), np.float32)
    bvp = np.zeros((1, 520), np.float32)
    for h in range(HEAD):
        wvpt[:, h * 65:h * 65 + 64] = Wv[h * 64:(h + 1) * 64, :].T
        bvp[0, h * 65:h * 65 + 64] = bv[h * 64:(h + 1) * 64]
        bvp[0, h * 65 + 64] = 1.0
    pos = (rel_h + rel_w).reshape(HEAD, D, N).astype(np.float16)
    return {
        "wzt": wzt.astype(np.float16),
        "bz": bz,
        "wvpt": wvpt.reshape(4, 128, 520).astype(np.float16),
        "bvp": bvp.astype(np.float16),
        "pos": pos,
    }


_CACHE = {}


def build_in_maps(x, Wq, bq, Wk, bk, Wv, bv, rel_h, rel_w):
    x = np.asarray(x, np.float32)
    consts = _prep_consts(
        *[np.asarray(a, np.float32) for a in (Wq, bq, Wk, bk, Wv, bv, rel_h, rel_w)]
    )
    xh = x.reshape(B, C, N).astype(np.float16)
    in_maps = []
    for c in range(N_CORES):
        m = dict(consts)
        m["xh"] = np.ascontiguousarray(xh[c * BPC:(c + 1) * BPC])
        in_maps.append(m)
    return in_maps


def kernel(x, Wq, bq, Wk, bk, Wv, bv, rel_h, rel_w, reg_qk, reg_v):
    # reg_qk / reg_v are computed-then-dropped by the reference -> unused.
    in_maps = build_in_maps(x, Wq, bq, Wk, bk, Wv, bv, rel_h, rel_w)

    if "nc" not in _CACHE:
        _CACHE["nc"] = build_bass()
    res = run_bass_kernel_spmd(_CACHE["nc"], in_maps, list(range(N_CORES)))
    outs = [np.asarray(r["out"]) for r in res.results]
    return np.concatenate(outs, axis=0).reshape(B, C, WD, HD)


if __name__ == "__main__":
    nc = build_bass()
    print("built ok")


# revision 33
# speedup vs baseline: 1.1121x; 1.0280x over previous
"""Trainium2 Bass kernel for nn_MHSA_40346922778634.

Math (per batch b, head h; the reference computes-then-drops the register
group, so reg_qk/reg_v are dead inputs):
  X = x[b] as [C=512, N=1024]
  Q = Wq X + bq ; K = Wk X + bk ; V = Wv X + bv       (per head: [64, N])
  P_h = (rel_h + rel_w) reshaped [head, 64, N]
  E[i,j] = Q_h[:,i].K_h[:,j] + P_h[:,i].Q_h[:,j]      ([N, N])
  attn = softmax(E, axis=-1)
  Out_h = V_h @ attn^T ; out[b, h*64:(h+1)*64] = Out_h + X[h*64:(h+1)*64]

Kernel strategy (8 cores, data-parallel over batch, 2 batches/core):
  - fp16 operands for projection + energy matmuls; bf16 for exp output
    (range) and AV matmuls.
  - Per head: z_h = [K_h; Q_h] produced DIRECTLY by the projection with a
    head-stacked weight layout (no partition-shuffle copies); u_h = [Q_h
    (DMA partition shift from z_h); P_h (DRAM)].
  - E^T = z^T u in one K=128 matmul per chunk; exp on ACT (the pacing
    engine at ~9.2us/head); AV = V_aug^T tt with a ones-column in vpt for
    the denominator row.
  - Softmax normalize: AV PSUM evacuated at once to SBUF (frees the PSUM
    slot fast -> avp bufs=1); denominator row packed [1,1024] -> [128,8]
    via DMA, reciprocal on all 128 DVE lanes (0.2us vs 6.5us), broadcast
    via DRAM staging + stride-0-partition DMA read (GpSimd
    partition_broadcast would swap Q7 libraries, ~7us/call), DVE multiply,
    GpSimd residual add (fp16 x).
  - Per-slot emission interleaves the lag-2 AV chunks + projections into
    the ACT-paced energy phase so the PE stream stays dense and warm;
    constants ride the scalar/gpsimd DMA queues so the sync queue and ACT
    queue are never head-of-line blocked.
"""

import sys

import numpy as np

try:
    import concourse.bass as bass  # noqa: F401
except Exception:  # pragma: no cover
    sys.path.insert(0, "/opt/trn_rl_repo")

import concourse.bass as bass  # noqa: F401
import concourse.tile as tile
from concourse import bacc, mybir
from concourse.bass_utils import run_bass_kernel_spmd

F32 = mybir.dt.float32
F16 = mybir.dt.float16
BF16 = mybir.dt.bfloat16
EXP = mybir.ActivationFunctionType.Exp

N_CORES = 8
B, C, WD, HD = 16, 512, 32, 32
HEAD, D, N = 8, 64, 1024
BPC = B // N_CORES  # batches per core
GH = BPC * HEAD  # global head slots per core


def build_bass():
    nc = bacc.Bacc("TRN2")

    xh_d = nc.dram_tensor("xh", [BPC, C, N], F16, kind="ExternalInput")
    wzt_d = nc.dram_tensor("wzt", [4, 128, 1024], F16, kind="ExternalInput")
    bz_d = nc.dram_tensor("bz", [HEAD, 128, 1], F32, kind="ExternalInput")
    wvpt_d = nc.dram_tensor("wvpt", [4, 128, 520], F16, kind="ExternalInput")
    bvp_d = nc.dram_tensor("bvp", [1, 520], F16, kind="ExternalInput")
    pos_d = nc.dram_tensor("pos", [HEAD, D, N], F16, kind="ExternalInput")
    out_d = nc.dram_tensor("out", [BPC, C, N], F32, kind="ExternalOutput")
    # per-slot staging row for the reciprocal broadcast (DMA replicate)
    r_d = nc.dram_tensor("r_stage", [GH, N], BF16, kind="Internal")

    with tile.TileContext(nc) as tc:
        with (
            tc.tile_pool(name="consts", bufs=1) as cpool,
            tc.tile_pool(name="work", bufs=2) as wpool,
            tc.tile_pool(name="psum", bufs=2, space="PSUM") as ppool,
        ):
            # ---- constants (scalar HWDGE queue; x goes on sync queue) ----
            wzt_sb = cpool.tile([128, 4, 1024], F16, name="wzt_sb")
            for kc in range(4):
                nc.scalar.dma_start(wzt_sb[:, kc, 0:256], wzt_d[kc, :, 0:256])
            for kc in range(4):
                nc.scalar.dma_start(wzt_sb[:, kc, 256:1024], wzt_d[kc, :, 256:1024])
            # bz/wvpt/bvt ride the (startup-idle) GpSimd SWDGE queue so they
            # don't head-of-line block the ACT queue in front of the first
            # exps (only sync/scalar/gpsimd can issue DMAs).
            bz_sb = cpool.tile([128, HEAD], F32, name="bz_sb")
            for h in range(HEAD):
                nc.gpsimd.dma_start(bz_sb[:, h:h + 1], bz_d[h])
            wvpt_sb = cpool.tile([128, 4, 520], F16, name="wvpt_sb")
            for kc in range(4):
                nc.gpsimd.dma_start(wvpt_sb[:, kc, :], wvpt_d[kc])
            # V bias broadcast to all partitions once; added on DVE during the
            # PSUM->SBUF copy instead of via ones-row matmuls.
            bvt_sb = cpool.tile([128, 520], F16, name="bvt_sb")
            nc.gpsimd.dma_start(bvt_sb[:], bvp_d[0:1, :].broadcast_to([128, 520]))
            # warm the ACT exp table during the initial DMAs
            warm_in = cpool.tile([1, 8], F32, name="warm_in")
            nc.vector.memset(warm_in[:], 0.0)
            warm_out = cpool.tile([1, 8], F32, name="warm_out")
            nc.scalar.activation(warm_out[:], warm_in[:], EXP)
            # u_all: per-head [Q_h(b); P_h] — P halves (partitions 64-127)
            # are batch-invariant, loaded once here.
            u_all = cpool.tile([128, HEAD, N], F16, name="u_all")
            for h in range(2):
                nc.sync.dma_start(u_all[64:128, h, :], pos_d[h])
            for h in range(2, HEAD):
                nc.gpsimd.dma_start(u_all[64:128, h, :], pos_d[h])

            def prep_x(b, x_sb=None, halves=(0, 1)):
                # DMA x on the sync queue (scalar-queue DMAs steal ACT time).
                if x_sb is None:
                    x_sb = wpool.tile([128, 4, N], F16, name=f"x_{b}", tag="x")
                for nh in halves:
                    for kc in range(4):
                        nc.sync.dma_start(
                            x_sb[:, kc, nh * 512:(nh + 1) * 512],
                            xh_d[b, kc * 128:(kc + 1) * 128, nh * 512:(nh + 1) * 512],
                        )
                return x_sb

            def emit_zproj_half(x_sb, h, z, nh):
                ps = ppool.tile([128, 512], F32, name=f"ps_z{h}{nh}", tag="pse", bufs=3)
                for kc in range(4):
                    nc.tensor.matmul(
                        ps[:],
                        wzt_sb[:, kc, h * 128:(h + 1) * 128],
                        x_sb[:, kc, nh * 512:(nh + 1) * 512],
                        start=(kc == 0),
                        stop=(kc == 3),
                    )
                nc.vector.tensor_scalar_add(
                    z[:, nh * 512:(nh + 1) * 512], ps[:], bz_sb[:, h:h + 1]
                )

            def emit_zproj(x_sb, h):
                # z_h = [K_h; Q_h] stacked on partitions, bias added, f16.
                z = wpool.tile([128, N], F16, name=f"z_{h}", tag="z", bufs=4)
                for nh in range(2):
                    emit_zproj_half(x_sb, h, z, nh)
                return z

            def emit_u(z, h):
                # u_h rows 0-63 = Q_h (partition-shift copy from z rows
                # 64-127); rows 64-127 (P_h) were loaded once at startup.
                nc.sync.dma_start(u_all[0:64, h, :], z[64:128, :])

            def emit_vproj(x_sb, vpt, c0, c1):
                # V^T padded projection (bf16); bias (incl. the ones column)
                # is added by the DVE during PSUM evacuation.
                for c8 in range(c0, c1):
                    ps = ppool.tile([128, 520], F32, name=f"ps_v{c8}", tag="pse", bufs=3)
                    for (lo, hi) in ((0, 512), (512, 520)):
                        for kc in range(4):
                            nc.tensor.matmul(
                                ps[:, lo:hi],
                                x_sb[:, kc, c8 * 128:(c8 + 1) * 128],
                                wvpt_sb[:, kc, lo:hi],
                                start=(kc == 0),
                                stop=(kc == 3),
                            )
                    nc.vector.tensor_add(vpt[:, c8, :], ps[:], bvt_sb[:])

            def emit_E_chunks(z, h, gh):
                # E^T chunks + exp; ACT paces this phase.
                tts = []
                for j in range(8):
                    eps = ppool.tile([128, N], F32, name=f"ps_e{gh}{j}", tag="pse", bufs=3)
                    for ih in range(2):
                        nc.tensor.matmul(
                            eps[:, ih * 512:(ih + 1) * 512],
                            z[:, j * 128:(j + 1) * 128],
                            u_all[:, h, ih * 512:(ih + 1) * 512],
                            start=True,
                            stop=True,
                        )
                    tt = wpool.tile([128, N], BF16, name=f"tt_{gh}_{j}", tag="tt", bufs=22)
                    nc.scalar.activation(tt[:], eps[:], EXP)
                    tts.append(tt)
                return tts

            def emit_AV(vpt, h, tts, gh):
                ops = ppool.tile([65, N], F32, name=f"ps_o{gh}", tag="avp", bufs=1)
                for j in range(8):
                    for mh in range(2):
                        nc.tensor.matmul(
                            ops[:, mh * 512:(mh + 1) * 512],
                            vpt[:, j, h * 65:h * 65 + 65],
                            tts[j][:, mh * 512:(mh + 1) * 512],
                            start=(j == 0),
                            stop=(j == 7),
                        )
                # evacuate PSUM immediately: frees the avp slot ~1.5us after
                # the burst so the next slot's AV never waits on the (long)
                # normalize chain; row 64 doubles as the denominator row.
                oc = wpool.tile([65, N], F32, name=f"oc_{gh}", tag="oc", bufs=4)
                nc.vector.tensor_copy(oc[:], ops[:])
                return oc

            def emit_norm_head(b, h, oc, gh):
                # DMA-only part of the normalize, emitted right after the AV
                # PSUM evacuation: residual prefetch + denominator pack.
                # Tail chains (gh>=13, after the last exp) alternate between
                # the sync and the now-idle scalar queue so the three final
                # chains pipeline instead of serializing on one HWDGE FIFO.
                eng = nc.scalar if gh >= 13 and gh % 2 == 0 else nc.sync
                xres = wpool.tile([64, N], F16, name=f"xres_{gh}", tag="xres", bufs=4)
                eng.dma_start(xres[:], xh_d[b, h * 64:(h + 1) * 64, :])
                dp = wpool.tile([128, 8], F32, name=f"dp_{gh}", tag="dp", bufs=3)
                eng.dma_start(dp[:], oc[64:65, :])
                return {"b": b, "h": h, "oc": oc, "xres": xres, "dp": dp,
                        "gh": gh, "eng": eng}

            def emit_norm_tail(st):
                # compute part, emitted a slot later so the DVE queue head
                # never waits on the pack DMA (which blocked the next slot's
                # bias-adds and stalled projection PSUM recycling).
                gh, b, h = st["gh"], st["b"], st["h"]
                rp = wpool.tile([128, 8], F32, name=f"rp_{gh}", tag="rp", bufs=3)
                nc.vector.reciprocal(rp[:], st["dp"][:])
                rpb = wpool.tile([128, 8], BF16, name=f"rpb_{gh}", tag="rpb", bufs=3)
                nc.vector.tensor_copy(rpb[:], rp[:])
                # broadcast 1/denom to 64 partitions via DRAM staging + a
                # stride-0-partition DMA read (GpSimd partition_broadcast
                # forces a Q7 library swap per call — ~7us stall)
                eng = st["eng"]
                eng.dma_start(r_d[gh:gh + 1, :], rpb[:])
                rps = wpool.tile([64, N], BF16, name=f"rps_{gh}", tag="rps", bufs=3)
                eng.dma_start(rps[:], r_d[gh:gh + 1, :].broadcast_to([64, N]))
                osb = wpool.tile([64, N], F32, name=f"osb_{gh}", tag="osb", bufs=4)
                nc.vector.tensor_mul(osb[:], st["oc"][0:64, :], rps[:])
                fin = wpool.tile([64, N], F32, name=f"fin_{gh}", tag="fin", bufs=4)
                nc.vector.tensor_add(fin[:], osb[:], st["xres"][:])
                eng.dma_start(out_d[b, h * 64:(h + 1) * 64, :], fin[:])

            # ---- software pipeline over GH=16 global head slots ----
            xs = {0: prep_x(0)}
            vpts = {0: wpool.tile([128, 8, 520], BF16, name="vpt_0", tag="vpt")}
            Z, TT, OPS = {}, {}, {}
            Z[0] = emit_zproj(xs[0], 0)
            emit_u(Z[0], 0)
            Z[1] = emit_zproj(xs[0], 1)
            emit_u(Z[1], 1)

            NORM = {}
            for gh in range(GH):
                b, h = divmod(gh, HEAD)
                z = Z.pop(gh)
                # normalize compute for slot gh-3 first: its DMAs landed a
                # slot ago, so the DVE queue head never blocks on them.
                if gh - 3 in NORM:
                    emit_norm_tail(NORM.pop(gh - 3))
                # Build PE "filler" units to interleave into the ACT-paced
                # energy phase so the PE never idles waiting on exp.
                fillers = []
                av_state = {}
                if gh >= 2:
                    b2, h2 = divmod(gh - 2, HEAD)
                    tts2 = TT.pop(gh - 2)
                    ops2 = ppool.tile(
                        [65, N], F32, name=f"ps_o{gh - 2}", tag="avp", bufs=1
                    )
                    av_state = {"ops": ops2, "b2": b2, "h2": h2}

                    def av_unit(j, ops2=ops2, vpt=vpts[b2], h2=h2, tts2=tts2):
                        for mh in range(2):
                            nc.tensor.matmul(
                                ops2[:, mh * 512:(mh + 1) * 512],
                                vpt[:, j, h2 * 65:h2 * 65 + 65],
                                tts2[j][:, mh * 512:(mh + 1) * 512],
                                start=(j == 0),
                                stop=(j == 7),
                            )

                    fillers += [lambda j=j: av_unit(j) for j in range(8)]
                if gh + 2 < GH:
                    b3, h3 = divmod(gh + 2, HEAD)
                    z3 = wpool.tile([128, N], F16, name=f"z_{gh + 2}", tag="z", bufs=4)
                    Z[gh + 2] = z3
                    fillers += [
                        lambda nh=nh, z3=z3, b3=b3, h3=h3: emit_zproj_half(
                            xs[b3], h3, z3, nh
                        )
                        for nh in range(2)
                    ]
                if gh < 2:
                    fillers += [
                        lambda c=c: emit_vproj(xs[0], vpts[0], c, c + 1)
                        for c in range(4 * gh, 4 * gh + 4)
                    ]
                elif 6 <= gh <= 9:
                    fillers += [
                        lambda c=c: emit_vproj(xs[1], vpts[1], c, c + 1)
                        for c in range(2 * (gh - 6), 2 * (gh - 6) + 2)
                    ]
                if gh == 4:
                    xs[1] = prep_x(1, halves=(0,))
                elif gh == 5:
                    prep_x(1, x_sb=xs[1], halves=(1,))
                    vpts[1] = wpool.tile([128, 8, 520], BF16, name="vpt_1", tag="vpt")

                # energy + exp for slot gh, fillers interleaved
                tts, fi = [], 0
                for j in range(8):
                    eps = ppool.tile([128, N], F32, name=f"ps_e{gh}{j}", tag="pse", bufs=3)
                    for ih in range(2):
                        nc.tensor.matmul(
                            eps[:, ih * 512:(ih + 1) * 512],
                            z[:, j * 128:(j + 1) * 128],
                            u_all[:, h, ih * 512:(ih + 1) * 512],
                            start=True,
                            stop=True,
                        )
                    tt = wpool.tile([128, N], BF16, name=f"tt_{gh}_{j}", tag="tt", bufs=22)
                    nc.scalar.activation(tt[:], eps[:], EXP)
                    tts.append(tt)
                    if j >= 1:
                        for _ in range(2):
                            if fi < len(fillers):
                                fillers[fi]()
                                fi += 1
                while fi < len(fillers):
                    fillers[fi]()
                    fi += 1
                TT[gh] = tts
                if gh + 2 < GH:
                    emit_u(Z[gh + 2], (gh + 2) % HEAD)
                if av_state:
                    ops2 = av_state["ops"]
                    oc = wpool.tile(
                        [65, N], F32, name=f"oc_{gh - 2}", tag="oc", bufs=4
                    )
                    nc.vector.tensor_copy(oc[:], ops2[:])
                    NORM[gh - 2] = emit_norm_head(
                        av_state["b2"], av_state["h2"], oc, gh - 2
                    )

            # epilogue: AV + norm for the last two slots; the final AV uses
            # a pse-tag PSUM tile (energy psum is drained by then) so it does
            # not serialize on the avp slot release.
            for gh, tag in ((GH - 2, "avp"), (GH - 1, "pse")):
                b2, h2 = divmod(gh, HEAD)
                tts2 = TT.pop(gh)
                ops2 = ppool.tile([65, N], F32, name=f"ps_o{gh}", tag=tag,
                                  bufs=1 if tag == "avp" else 3)
                for j in range(8):
                    for mh in range(2):
                        nc.tensor.matmul(
                            ops2[:, mh * 512:(mh + 1) * 512],
                            vpts[b2][:, j, h2 * 65:h2 * 65 + 65],
                            tts2[j][:, mh * 512:(mh + 1) * 512],
                            start=(j == 0),
                            stop=(j == 7),
                        )
                oc = wpool.tile([65, N], F32, name=f"oc_{gh}", tag="oc", bufs=4)
                nc.vector.tensor_copy(oc[:], ops2[:])
                NORM[gh] = emit_norm_head(b2, h2, oc, gh)
                if gh - 1 in NORM:
                    emit_norm_tail(NORM.pop(gh - 1))
            for k in sorted(NORM):
                emit_norm_tail(NORM.pop(k))

    nc.compile()
    return nc


def _prep_consts(Wq, bq, Wk, bk, Wv, bv, rel_h, rel_w):
    WkT = np.ascontiguousarray(Wk.T).reshape(4, 128, 512)
    WqT = np.ascontiguousarray(Wq.T).reshape(4, 128, 512)
    wzt = np.empty((4, 128, 1024), np.float32)
    bz = np.empty((HEAD, 128, 1), np.float32)
    for h in range(HEAD):
        wzt[:, :, h * 128:h * 128 + 64] = WkT[:, :, h * 64:(h + 1) * 64]
        wzt[:, :, h * 128 + 64:h * 128 + 128] = WqT[:, :, h * 64:(h + 1) * 64]
        bz[h, 0:64, 0] = bk[h * 64:(h + 1) * 64]
        bz[h, 64:128, 0] = bq[h * 64:(h + 1) * 64]
    wvpt = np.zeros((512, 520), np.float32)
    bvp = np.zeros((1, 520), np.float32)
    for h in range(HEAD):
        wvpt[:, h * 65:h * 65 + 64] = Wv[h * 64:(h + 1) * 64, :].T
        bvp[0, h * 65:h * 65 + 64] = bv[h * 64:(h + 1) * 64]
        bvp[0, h * 65 + 64] = 1.0
    pos = (rel_h + rel_w).reshape(HEAD, D, N).astype(np.float16)
    return {
        "wzt": wzt.astype(np.float16),
        "bz": bz,
        "wvpt": wvpt.reshape(4, 128, 520).astype(np.float16),
        "bvp": bvp.astype(np.float16),
        "pos": pos,
    }


_CACHE = {}


def build_in_maps(x, Wq, bq, Wk, bk, Wv, bv, rel_h, rel_w):
    x = np.asarray(x, np.float32)
    consts = _prep_consts(
        *[np.asarray(a, np.float32) for a in (Wq, bq, Wk, bk, Wv, bv, rel_h, rel_w)]
    )
    xh = x.reshape(B, C, N).astype(np.float16)
    in_maps = []
    for c in range(N_CORES):
        m = dict(consts)
        m["xh"] = np.ascontiguousarray(xh[c * BPC:(c + 1) * BPC])
        in_maps.append(m)
    return in_maps


def kernel(x, Wq, bq, Wk, bk, Wv, bv, rel_h, rel_w, reg_qk, reg_v):
    # reg_qk / reg_v are computed-then-dropped by the reference -> unused.
    in_maps = build_in_maps(x, Wq, bq, Wk, bk, Wv, bv, rel_h, rel_w)

    if "nc" not in _CACHE:
        _CACHE["nc"] = build_bass()
    res = run_bass_kernel_spmd(_CACHE["nc"], in_maps, list(range(N_CORES)))
    outs = [np.asarray(r["out"]) for r in res.results]
    return np.concatenate(outs, axis=0).reshape(B, C, WD, HD)


if __name__ == "__main__":
    nc = build_bass()
    print("built ok")
